# revision 8
# baseline (speedup 1.0000x reference)
"""Trainium2 Bass kernel for nn_CombineNode_7395933684091 (gnn_message_passing).

Hierarchy: 128 leaf terms (each D=1024 -> H=32), 16 internal terms
(concat of 8 children hiddens, 256 -> 32), 1 root (concat of 16
internal hiddens, 512 -> 32); every term also has a 1-dim predict head.
All matmuls followed by tanh.

Strategy: data-parallel over batch across 8 cores (Bc = 1024 rows per
core), weights replicated. On-chip layout keeps hidden features on the
PARTITION axis ("h^T layout": tiles are [features, batch]), so every
level's contraction is a natural PE matmul and the child-concat is just
stacking partition tiles. x and all weights are repacked on the host so
every DMA is contiguous per partition.

Leaf level: 4 panels x 8 groups (4 leaves) x 8 k-chunk accumulated
[128,128]x[128,512] matmuls. The per-term predict heads ride along as
extra block-diagonal columns fused into the internal-level stationary
operand (cw) and the root-level stationary operand (rw2), so they cost
no extra PE streaming.

Matmul operands are float16: same PE stream rate as f32r (1 col/cycle)
but enables Fast Weight Load (fp32 disables FWL) so LDWEIGHTS hides
behind the matmul stream, and halves HBM + SBUF traffic. fp16's 10
mantissa bits keep the end-to-end max abs error ~1.6e-3 (vs 2e-2 gate).

v2 scheduling notes (from perfetto analysis of v1 @153.0us):
- PE pre-warm uses the bf16 const AP (fp32 forces LOW_HIGH
  2-pass matmuls) and is sized to end when the first x/weight chunks
  land (~9.3us), not overshoot to 11.8us.
- Every dma_start costs ~600ns of ISSUE time on its engine, and a
  1KB-run 128KB transfer sustains only ~140GB/s per queue, so the
  preamble is paced: scalar = cc + 8 x-bn0 chunks only (it must be free
  for leaf tanhs by ~16us); x-bn1 / cw issues are deferred into the
  stream emission. sync = wave1 + wave2 weight chunks + rw2 + panels.
- Combine matmuls are deferred by half a leaf group (a FIFO popped
  twice per group) so the leaf-tanh latency (~460ns) never stalls PE.
- Endgame runs the last internal node in 4 quarter-pipes (128 cols)
  with each quarter's root chain emitted under the next quarter's leaf
  matmuls; final flushes are split across the sync and scalar queues.
"""

import numpy as np

B, D, H = 8192, 1024, 32
L, I, CPI = 128, 16, 8
NCORES = 8
BC = B // NCORES      # 1024 batch rows per core
BN = 512              # batch tile width (one PSUM bank of f32)
NBH = BC // BN        # 2 batch halves
KC = D // 128         # 8 contraction chunks for the leaf level
NPANEL = 4            # leaf panels (8 groups of 4 leaves each)
GPP = 8               # groups per panel
NOUT = L + I + 1      # 145
NWARM = 6             # pre-warm matmuls (512 cols each)

MM_DT = "float16"

_CACHE = {}


def _build_nc():
    from contextlib import ExitStack

    import concourse.mybir as mybir
    import concourse.tile as tile
    from concourse import bacc

    f32 = mybir.dt.float32
    bf16 = mybir.dt.bfloat16
    Tanh = mybir.ActivationFunctionType.Tanh
    mmdt = getattr(mybir.dt, MM_DT)

    nc = bacc.Bacc("TRN2", target_bir_lowering=False, debug=False)

    xt = nc.dram_tensor("xt", [D, BC], mmdt, kind="ExternalInput")
    # leaf weights, panel-major: lwh[p, pp, k*1024 + j] so each panel is
    # one contiguous [128, 8K] DMA (16KB/partition runs)
    lwh = nc.dram_tensor("lwh", [NPANEL, 128, KC * 1024], mmdt, kind="ExternalInput")
    # fused internal-trans + leaf-predict stationary: per (node i, chunk j)
    # a [128, 128] block: cols 0:32 int_W chunk, col 32+4j+c leaf Wp diag,
    # rest zero padding (full-width stationaries keep LDW pull-ahead alive)
    cw = nc.dram_tensor("cw", [128, I * 2 * 128], mmdt, kind="ExternalInput")
    # fused root-trans + int-predict stationary: per panel q a [128, 128]
    # block (cols 0:32 root_W chunk, 32:48 int Wp diag, rest zero); block 4
    # holds root_Wp in rows 0:32 of col 0 (padded to 128 wide so the LDW
    # pull-ahead isn't blocked by a narrow stationary)
    rw2 = nc.dram_tensor("rw2", [128, (NPANEL + 1) * 128], mmdt, kind="ExternalInput")
    # all f32 per-partition bias constants in one tensor:
    # cols 0:32 leaf_b, 32:36 int_b, 36:52 leaf_bp (rows 0:8),
    # 52 int_bp (rows 0:16), 53 root_b (rows 0:32), 54 root_bp (row 0)
    cc = nc.dram_tensor("cc", [128, 55], f32, kind="ExternalInput")
    # fp16 output staging: predictions are tanh outputs in [-1,1], so the
    # ~5e-4 fp16 quantization is well inside the error budget; halves the
    # final store drain. Host upcasts to f32.
    out = nc.dram_tensor("out", [NOUT, BC], mmdt, kind="ExternalOutput")

    mm = nc.tensor.matmul

    with tile.TileContext(nc) as tc, ExitStack() as ctx:
        consts = ctx.enter_context(tc.tile_pool(name="consts", bufs=1))
        wpool = ctx.enter_context(tc.tile_pool(name="wpool", bufs=4))
        work = ctx.enter_context(tc.tile_pool(name="work", bufs=18))
        keep = ctx.enter_context(tc.tile_pool(name="keep", bufs=1))
        psum = ctx.enter_context(tc.tile_pool(name="psum", bufs=1, space="PSUM"))

        # --- PE pre-warm: unthrottles the HAM clock gate (PE boots at
        # 1.2 GHz; ~3.4us of sustained activity -> 2.4 GHz). bf16 const
        # APs (preloaded) keep it to one MATMUL per mm (fp32 would run
        # LOW_HIGH 2-pass) and nothing gates the first one.
        warm_st = nc.const_aps.tensor(1.0, (128, 128), bf16)
        warm_mv = nc.const_aps.tensor(1.0, (128, BN), bf16)
        pwarm = psum.tile([128, BN], f32, tag="misc", bufs=1, name="pwarm")
        for _ in range(NWARM):
            mm(pwarm[:], warm_st, warm_mv, start=True, stop=True,
               skip_group_check=True)

        # --- preamble DMA issues. scalar: cc + x bn0 chunks ONLY (the
        # engine must be free for leaf tanhs by ~16us; each issue burns
        # ~600ns). sync: wave1 (leaf groups 0-3 per k), wave2 (groups
        # 4-7 per k), rw2, then whole panels 1-3 (16KB/partition runs).
        cc_sb = consts.tile([128, 55], f32, name="cc_sb")
        nc.scalar.dma_start(cc_sb[:], cc[:])

        xt_sb = consts.tile([128, KC * BC], mmdt, name="xt_sb")
        wp0 = wpool.tile([128, KC * 1024], mmdt, tag="wpanel", name="wp0")
        for k in range(KC):
            nc.scalar.dma_start(
                xt_sb[:, k * BC:k * BC + BN], xt[k * 128:(k + 1) * 128, 0:BN]
            )
        for k in range(KC):
            nc.sync.dma_start(
                wp0[:, k * 1024:k * 1024 + 512],
                lwh[0, :, k * 1024:k * 1024 + 512],
            )
        for k in range(KC):
            nc.sync.dma_start(
                wp0[:, k * 1024 + 512:(k + 1) * 1024],
                lwh[0, :, k * 1024 + 512:(k + 1) * 1024],
            )
        rw2_sb = consts.tile([128, (NPANEL + 1) * 128], mmdt, name="rw2_sb")
        nc.sync.dma_start(rw2_sb[:], rw2[:])
        wps = {0: wp0}
        for q in (1, 2, 3):
            wps[q] = wpool.tile([128, KC * 1024], mmdt, tag="wpanel", name=f"wp{q}")
            nc.sync.dma_start(wps[q][:], lwh[q])
        cw_sb = consts.tile([128, I * 2 * 128], mmdt, name="cw_sb")

        # scalar-engine warm: force the tanh ACT table load during the DMA
        # preamble instead of on the first real activation
        act_warm = work.tile([1, 1], f32, tag="actw", bufs=1, name="act_warm")
        nc.scalar.activation(act_warm[:], pwarm[0:1, 0:1], Tanh)

        # leaf predicts: node i at cols i*BC (+bn*BN); flushed per panel
        lp_sb = keep.tile([8, I * BC], mmdt, name="lp_sb")
        intp_sb = keep.tile([16, BC], mmdt, name="intp_sb")
        rootp_sb = keep.tile([1, BC], mmdt, name="rootp_sb")

        inth = {}      # (panel, bn) -> [128, BN] tile: nodes 4p..4p+3 h^T
        prc1 = psum.tile([128, BN], f32, tag="prcinc", bufs=1, name="prc1")

        # deferred-op FIFO: each entry emits one PE-consuming op (a comb
        # matmul, a root contraction, a flush). Popped twice per leaf
        # group (after the 4th and 8th k-matmul) so producers' tanh
        # latency is always covered by >=0.85us of leaf streaming.
        fifo = []

        def pop_slot():
            if fifo:
                fifo.pop(0)()

        def leaf_mm(wp, gl, k, bn, pg, cols=None):
            c0 = bn * BN if cols is None else cols[0]
            cw_ = BN if cols is None else cols[1]
            mm(
                pg[:],
                wp[:, k * 1024 + gl * 128:k * 1024 + (gl + 1) * 128],
                xt_sb[:, k * BC + c0:k * BC + c0 + cw_],
                start=(k == 0),
                stop=(k == KC - 1),
            )

        def leaf_tanh(p, gl, bn, pg):
            lh = work.tile([128, BN], mmdt, tag="lh", name=f"lh{p}{bn}{gl}")
            nc.scalar.activation(
                lh[:], pg[:], Tanh, bias=cc_sb[:, GPP * p + gl:GPP * p + gl + 1]
            )
            return lh

        def comb_mm(p, il, j, lh, pcomb):
            """Fused internal-trans + leaf-predict matmul.

            pcomb rows 0:32 accumulate node (4p+il)'s hidden
            pre-activation over its two child groups; rows 32:40 pick up
            the group's 4 leaf predict dots via the block-diagonal
            columns (zeros elsewhere)."""
            i = 4 * p + il
            mm(
                pcomb[:],
                cw_sb[:, (2 * i + j) * 128:(2 * i + j + 1) * 128],
                lh[:],
                start=(j == 0),
                stop=(j == 1),
                skip_group_check=True,
            )

        def comb_post(p, il, bn, ith, pcomb):
            i = 4 * p + il
            nc.scalar.activation(
                ith[32 * il:32 * il + 32, :],
                pcomb[0:32, :],
                Tanh,
                bias=cc_sb[32 * il:32 * il + 32, 32 + p:33 + p],
            )
            nc.scalar.activation(
                lp_sb[:, i * BC + bn * BN:i * BC + bn * BN + BN],
                pcomb[32:40, :], Tanh, bias=cc_sb[0:8, 36 + i:37 + i],
            )

        def flush_lp(p, bn=None, irange=(0, 4)):
            i0, i1 = irange
            ni = i1 - i0
            if bn is None:
                nc.sync.dma_start(
                    out[32 * p + 8 * i0:32 * p + 8 * i1, :].rearrange(
                        "(i v) c -> v i c", v=8
                    ),
                    lp_sb[:, (4 * p + i0) * BC:(4 * p + i1) * BC].rearrange(
                        "v (i c) -> v i c", c=BC
                    ),
                )
            else:
                nc.sync.dma_start(
                    out[32 * p + 8 * i0:32 * p + 8 * i1,
                        bn * BN:bn * BN + BN].rearrange("(i v) c -> v i c", v=8),
                    lp_sb[:].rearrange("v (i c) -> v i c", c=BC)[
                        :, 4 * p + i0:4 * p + i1, bn * BN:bn * BN + BN
                    ],
                )

        def root_mm_inc(p, ith):
            """One panel's contribution to the bn=1 root/int-predict
            contraction, accumulated across panels in a persistent bank."""
            mm(
                prc1[:],
                rw2_sb[:, 128 * p:128 * (p + 1)],
                ith[:],
                start=(p == 0),
                stop=(p == NPANEL - 1),
                skip_group_check=True,
            )

        def root_post_mm(bn, prc, rh):
            prp = psum.tile([128, BN], f32, tag="pcomb", bufs=2,
                            name=f"prp{bn}")
            mm(prp[0:128, :], rw2_sb[0:32, NPANEL * 128:(NPANEL + 1) * 128],
               rh[:], start=True, stop=True, skip_group_check=True)
            nc.scalar.activation(
                rootp_sb[0:1, bn * BN:bn * BN + BN], prp[0:1, :], Tanh,
                bias=cc_sb[0:1, 54:55],
            )
            nc.scalar.activation(
                intp_sb[:, bn * BN:bn * BN + BN], prc[32:48, :], Tanh,
                bias=cc_sb[0:16, 52:53],
            )

        # ---- comb closures --------------------------------------------
        pcombs = {}

        def make_comb(p, il, j, bn, lh, ith):
            def run():
                if j == 0:
                    pcombs[(p, il, bn)] = psum.tile(
                        [128, BN], f32, tag="pcomb", bufs=2,
                        name=f"pc{p}{bn}{il}")
                pc = pcombs[(p, il, bn)]
                comb_mm(p, il, j, lh, pc)
                if j == 1:
                    comb_post(p, il, bn, ith, pc)
            return run

        # --- panel 0: pure leaf streams first, k-outer waves for bn=0 so
        # matmuls chase the arriving x/weight chunks. No FIFO pops before
        # bn=1 (cw hasn't landed). The remaining scalar-queue DMA issues
        # (x bn1, cw) are emitted between the tanh batches so the scalar
        # engine's issue backlog never delays a data-gated tanh, and
        # every chunk is issued before any matmul that consumes it.
        def xt_bn1_issue(ks):
            for k in ks:
                nc.scalar.dma_start(
                    xt_sb[:, k * BC + BN:(k + 1) * BC],
                    xt[k * 128:(k + 1) * 128, BN:BC],
                )

        ith00 = keep.tile([128, BN], mmdt, tag="inth00", name="inth00")
        for g0 in (0, 4):
            pgs = [
                psum.tile([128, BN], f32, tag="pg", bufs=4, name=f"pgko{g0}{q}")
                for q in range(4)
            ]
            for k in range(KC):
                for q in range(4):
                    leaf_mm(wp0, g0 + q, k, 0, pgs[q])
            if g0 == 0:
                xt_bn1_issue((0, 1, 2, 3))
            else:
                xt_bn1_issue((4, 5, 6, 7))
                nc.scalar.dma_start(cw_sb[:, 0:1024], cw[:, 0:1024])
            for q in range(4):
                gl = g0 + q
                lh = leaf_tanh(0, gl, 0, pgs[q])
                fifo.append(make_comb(0, gl // 2, gl % 2, 0, lh, ith00))
            if g0 == 4:
                nc.scalar.dma_start(cw_sb[:, 1024:I * 2 * 128],
                                    cw[:, 1024:I * 2 * 128])
        inth[(0, 0)] = ith00

        ith01 = keep.tile([128, BN], mmdt, tag="inth01", name="inth01")
        for gl in range(GPP):
            pg = psum.tile([128, BN], f32, tag="pg", bufs=4, name=f"pg0b{gl}")
            for k in range(4):
                leaf_mm(wp0, gl, k, 1, pg)
            pop_slot()
            for k in range(4, KC):
                leaf_mm(wp0, gl, k, 1, pg)
            pop_slot()
            lh = leaf_tanh(0, gl, 1, pg)
            fifo.append(make_comb(0, gl // 2, gl % 2, 1, lh, ith01))
        inth[(0, 1)] = ith01

        fifo.append(lambda: root_mm_inc(0, inth[(0, 1)]))
        fifo.append(lambda: flush_lp(0))

        # --- panels 1..3. Panel 3 runs bn=1 first so its root chain
        # (incremental prc1) closes while bn=0's leaf stream still runs.
        prc0 = {}

        def prc0_partial():
            prc0["t"] = psum.tile([128, BN], f32, tag="misc", bufs=1,
                                  name="prc0")
            for q in range(NPANEL - 1):
                mm(
                    prc0["t"][:],
                    rw2_sb[:, 128 * q:128 * (q + 1)],
                    inth[(q, 0)][:],
                    start=(q == 0),
                    stop=False,
                    skip_group_check=True,
                )

        for p in range(1, NPANEL):
            wp = wps[p]
            bns = (1, 0) if p == NPANEL - 1 else (0, 1)
            for bn in bns:
                ith = keep.tile([128, BN], mmdt, tag=f"inth{p}{bn}",
                                name=f"inth{p}{bn}")
                ngl = 6 if (p == NPANEL - 1 and bn == 0) else 8
                for gl in range(ngl):
                    pg = psum.tile([128, BN], f32, tag="pg", bufs=4,
                                   name=f"pg{p}{bn}{gl}")
                    for k in range(4):
                        leaf_mm(wp, gl, k, bn, pg)
                    pop_slot()
                    for k in range(4, KC):
                        leaf_mm(wp, gl, k, bn, pg)
                    pop_slot()
                    lh = leaf_tanh(p, gl, bn, pg)
                    fifo.append(make_comb(p, gl // 2, gl % 2, bn, lh, ith))
                inth[(p, bn)] = ith

                if p < NPANEL - 1:
                    if bn == 1:
                        fifo.append(
                            lambda p=p: root_mm_inc(p, inth[(p, 1)])
                        )
                        fifo.append(lambda p=p: flush_lp(p))
                elif bn == 1:
                    # close bn1's root chain + store its halves, then open
                    # bn0's root contraction (panels 0..2 ready now). Split
                    # into one-PE-op closures so each tanh's latency hides
                    # under half a leaf group of streaming.
                    rh1 = work.tile([32, BN], mmdt, tag="rh", bufs=2,
                                    name="rh1")

                    def close_a():
                        root_mm_inc(NPANEL - 1, inth[(NPANEL - 1, 1)])

                    def close_b():
                        nc.scalar.activation(rh1[:], prc1[0:32, :], Tanh,
                                             bias=cc_sb[0:32, 53:54])

                    def close_c():
                        root_post_mm(1, prc1, rh1)
                        flush_lp(NPANEL - 1, 1)

                    def close_d():
                        nc.sync.dma_start(
                            out[L:L + I, BN:BC], intp_sb[:, BN:BC]
                        )
                    fifo.append(close_a)
                    fifo.append(close_b)
                    fifo.append(close_c)
                    fifo.append(close_d)
                    fifo.append(prc0_partial)
                else:
                    # endgame: node 15 (groups 6,7) runs as 4 quarter
                    # pipelines (128 cols); each quarter's root chain is
                    # emitted under the next quarter's leaf matmuls so
                    # the final serial chain is one quarter wide.
                    i3 = 4 * p + 3
                    QB = BN // 4
                    rh = work.tile([32, BN], mmdt, tag="rh", bufs=2,
                                   name="rh0s")
                    prp = psum.tile([128, BN], f32, tag="prcinc", bufs=1,
                                    name="prp0s")
                    qcombs = {}
                    lhq = {}

                    def q_leaf(qi, j):
                        cs = qi * QB
                        gl = 6 + j
                        pg = psum.tile([128, QB], f32, tag="pg", bufs=4,
                                       name=f"pgq{qi}{j}")
                        for k in range(KC):
                            leaf_mm(wp, gl, k, 0, pg, cols=(cs, QB))
                        lh = work.tile([128, QB], mmdt, tag="lh",
                                       name=f"lhq{qi}{j}")
                        nc.scalar.activation(
                            lh[:], pg[:], Tanh,
                            bias=cc_sb[:, GPP * p + gl:GPP * p + gl + 1],
                        )
                        lhq[(qi, j)] = lh

                    def q_comb(qi):
                        cs = qi * QB
                        qcombs[qi] = psum.tile([128, QB], f32, tag="pcomb",
                                               bufs=2, name=f"pcq{qi}")
                        for j in range(2):
                            mm(
                                qcombs[qi][:],
                                cw_sb[:, (2 * i3 + j) * 128:(2 * i3 + j + 1) * 128],
                                lhq.pop((qi, j))[:],
                                start=(j == 0),
                                stop=(j == 1),
                                skip_group_check=True,
                            )
                        nc.scalar.activation(
                            ith[96:128, cs:cs + QB], qcombs[qi][0:32, :],
                            Tanh, bias=cc_sb[96:128, 32 + p:33 + p],
                        )
                        nc.scalar.activation(
                            lp_sb[:, i3 * BC + cs:i3 * BC + cs + QB],
                            qcombs[qi][32:40, :], Tanh,
                            bias=cc_sb[0:8, 36 + i3:37 + i3],
                        )

                    def q_rootA(qi):
                        cs = qi * QB
                        mm(
                            prc0["t"][:, cs:cs + QB],
                            rw2_sb[:, 128 * (NPANEL - 1):128 * NPANEL],
                            ith[:, cs:cs + QB],
                            start=False,
                            stop=True,
                            skip_group_check=True,
                        )
                        nc.scalar.activation(
                            rh[:, cs:cs + QB], prc0["t"][0:32, cs:cs + QB],
                            Tanh, bias=cc_sb[0:32, 53:54],
                        )

                    def q_rootB(qi):
                        cs = qi * QB
                        mm(prp[0:128, cs:cs + QB],
                           rw2_sb[0:32, NPANEL * 128:(NPANEL + 1) * 128],
                           rh[:, cs:cs + QB],
                           start=True, stop=True, skip_group_check=True)
                        nc.scalar.activation(
                            rootp_sb[0:1, cs:cs + QB], prp[0:1, cs:cs + QB],
                            Tanh, bias=cc_sb[0:1, 54:55],
                        )
                        nc.scalar.activation(
                            intp_sb[:, cs:cs + QB],
                            prc0["t"][32:48, cs:cs + QB],
                            Tanh, bias=cc_sb[0:16, 52:53],
                        )

                    # quarter pipeline: chain ops of quarter qi ride under
                    # quarter qi+1's leaf streams
                    q_leaf(0, 0)
                    pop_slot()          # drains comb(il2) -> then flush
                    q_leaf(0, 1)
                    pop_slot()
                    q_comb(0)
                    q_leaf(1, 0)
                    flush_lp(NPANEL - 1, 0, irange=(0, 3))
                    q_rootA(0)
                    q_leaf(1, 1)
                    q_rootB(0)
                    q_comb(1)
                    q_leaf(2, 0)
                    q_rootA(1)
                    q_leaf(2, 1)
                    q_rootB(1)
                    q_comb(2)
                    q_leaf(3, 0)
                    q_rootA(2)
                    q_leaf(3, 1)
                    q_rootB(2)
                    q_comb(3)
                    q_rootA(3)
                    q_rootB(3)
                    # final flushes, split across both DMA queues
                    flush_lp(NPANEL - 1, 0, irange=(3, 4))
                    nc.scalar.dma_start(out[L:L + I, 0:BN], intp_sb[:, 0:BN])
                    nc.sync.dma_start(out[L + I:NOUT, :], rootp_sb[:])

    nc.compile()
    return nc


def _pack_weights(inp):
    f = np.float32
    f16 = np.float16
    leaf_b = np.asarray(inp["leaf_b"], f)
    int_W = np.asarray(inp["int_W"], f)
    int_b = np.asarray(inp["int_b"], f)
    root_W = np.asarray(inp["root_W"], f)
    root_b = np.asarray(inp["root_b"], f)
    leaf_Wp = np.asarray(inp["leaf_Wp"], f)
    leaf_bp = np.asarray(inp["leaf_bp"], f)
    int_Wp = np.asarray(inp["int_Wp"], f)
    int_bp = np.asarray(inp["int_bp"], f)
    root_Wp = np.asarray(inp["root_Wp"], f)
    root_bp = np.asarray(inp["root_bp"], f)

    w = {}
    lw = np.asarray(inp["leaf_W"], f16).transpose(1, 0, 2).reshape(D, L * H)
    w["lwh"] = np.ascontiguousarray(
        lw.reshape(KC, 128, NPANEL, 1024).transpose(2, 1, 0, 3).reshape(
            NPANEL, 128, KC * 1024
        )
    )

    cw = np.zeros((128, I * 2 * 128), f16)
    for i in range(I):
        for j in range(2):
            base = (2 * i + j) * 128
            # int_W chunk j of node i: rows (c*32+h) = child (4j+c) hidden h
            cw[:, base:base + 32] = int_W[i, 128 * j:128 * (j + 1), :]
            for c in range(4):
                lv = 8 * i + 4 * j + c
                cw[c * 32:(c + 1) * 32, base + 32 + 4 * j + c] = leaf_Wp[lv, :, 0]
    w["cw"] = cw

    rw2 = np.zeros((128, (NPANEL + 1) * 128), f16)
    for q in range(NPANEL):
        rw2[:, 128 * q:128 * q + 32] = root_W[128 * q:128 * (q + 1), :]
        for c in range(4):
            iv = 4 * q + c
            rw2[c * 32:(c + 1) * 32, 128 * q + 32 + 4 * q + c] = int_Wp[iv, :, 0]
    rw2[0:32, NPANEL * 128] = root_Wp[:, 0]
    w["rw2"] = rw2

    cc = np.zeros((128, 55), f)
    cc[:, 0:32] = leaf_b.reshape(32, 128).T       # leaf biases, col=h, part=leaf%...
    cc[:, 32:36] = int_b.reshape(4, 128).T
    cc[0:8, 36:52] = leaf_bp.reshape(16, 8).T
    cc[0:16, 52] = int_bp[:, 0]
    cc[0:32, 53] = root_b
    cc[0, 54] = root_bp[0]
    w["cc"] = cc
    return w


def kernel(**inputs):
    from concourse.bass_utils import run_bass_kernel_spmd

    nc = _CACHE.get("nc")
    if nc is None:
        nc = _CACHE["nc"] = _build_nc()

    x = np.asarray(inputs["x"], np.float32)
    w = _pack_weights(inputs)
    in_maps = []
    for c in range(NCORES):
        m = dict(w)
        m["xt"] = np.ascontiguousarray(x[c * BC:(c + 1) * BC, :].T.astype(np.float16))
        in_maps.append(m)

    res = run_bass_kernel_spmd(nc, in_maps, core_ids=list(range(NCORES)))
    _CACHE["last_res"] = res
    outs = [res.results[c]["out"] for c in range(NCORES)]
    full = np.concatenate([o[:, :, None] for o in outs], axis=1)  # [145, B, 1]
    return full.astype(np.float32)


# revision 11
# speedup vs baseline: 1.0801x; 1.0801x over previous
"""Trainium2 Bass kernel for nn_CombineNode_7395933684091 (gnn_message_passing).

Hierarchy: 128 leaf terms (each D=1024 -> H=32), 16 internal terms
(concat of 8 children hiddens, 256 -> 32), 1 root (concat of 16
internal hiddens, 512 -> 32); every term also has a 1-dim predict head.
All matmuls followed by tanh.

Strategy: data-parallel over batch across 8 cores (Bc = 1024 rows per
core), weights replicated. On-chip layout keeps hidden features on the
PARTITION axis ("h^T layout": tiles are [features, batch]), so every
level's contraction is a natural PE matmul and the child-concat is just
stacking partition tiles. x and all weights are repacked on the host so
every DMA is contiguous per partition.

Leaf level: 4 panels x 8 groups (4 leaves) x 8 k-chunk accumulated
[128,128]x[128,512] matmuls. The per-term predict heads ride along as
extra block-diagonal columns fused into the internal-level stationary
operand (cw) and the root-level stationary operand (rw2), so they cost
no extra PE streaming.

Matmul operands are float16: same PE stream rate as f32r (1 col/cycle)
but enables Fast Weight Load (fp32 disables FWL) so LDWEIGHTS hides
behind the matmul stream, and halves HBM + SBUF traffic. fp16's 10
mantissa bits keep the end-to-end max abs error ~1.6e-3 (vs 2e-2 gate).

v2 scheduling notes (from perfetto analysis of v1 @153.0us):
- PE pre-warm uses the bf16 const AP (fp32 forces LOW_HIGH
  2-pass matmuls) and is sized to end when the first x/weight chunks
  land (~9.3us), not overshoot to 11.8us.
- Every dma_start costs ~600ns of ISSUE time on its engine, and a
  1KB-run 128KB transfer sustains only ~140GB/s per queue, so the
  preamble is paced: scalar = cc + 8 x-bn0 chunks only (it must be free
  for leaf tanhs by ~16us); x-bn1 / cw issues are deferred into the
  stream emission. sync = wave1 + wave2 weight chunks + rw2 + panels.
- Combine matmuls are deferred by half a leaf group (a FIFO popped
  twice per group) so the leaf-tanh latency (~460ns) never stalls PE.
- Endgame runs the last internal node in 4 quarter-pipes (128 cols)
  with each quarter's root chain emitted under the next quarter's leaf
  matmuls; final flushes are split across the sync and scalar queues.
"""

import numpy as np

B, D, H = 8192, 1024, 32
L, I, CPI = 128, 16, 8
NCORES = 8
BC = B // NCORES      # 1024 batch rows per core
BN = 512              # batch tile width (one PSUM bank of f32)
NBH = BC // BN        # 2 batch halves
KC = D // 128         # 8 contraction chunks for the leaf level
NPANEL = 4            # leaf panels (8 groups of 4 leaves each)
GPP = 8               # groups per panel
NOUT = L + I + 1      # 145
NWARM = 6             # pre-warm matmuls (512 cols each)

MM_DT = "float16"

_CACHE = {}


def _build_nc():
    from contextlib import ExitStack

    import concourse.mybir as mybir
    import concourse.tile as tile
    from concourse import bacc

    f32 = mybir.dt.float32
    bf16 = mybir.dt.bfloat16
    Tanh = mybir.ActivationFunctionType.Tanh
    mmdt = getattr(mybir.dt, MM_DT)

    nc = bacc.Bacc("TRN2", target_bir_lowering=False, debug=False)

    xt = nc.dram_tensor("xt", [D, BC], mmdt, kind="ExternalInput")
    # leaf weights, panel-major: lwh[p, pp, k*1024 + j] so each panel is
    # one contiguous [128, 8K] DMA (16KB/partition runs)
    lwh = nc.dram_tensor("lwh", [NPANEL, 128, KC * 1024], mmdt, kind="ExternalInput")
    # fused internal-trans + leaf-predict stationary: per (node i, chunk j)
    # a [128, 128] block: cols 0:32 int_W chunk, col 32+4j+c leaf Wp diag,
    # rest zero padding (full-width stationaries keep LDW pull-ahead alive)
    cw = nc.dram_tensor("cw", [128, I * 2 * 128], mmdt, kind="ExternalInput")
    # fused root-trans + int-predict stationary: per panel q a [128, 128]
    # block (cols 0:32 root_W chunk, 32:48 int Wp diag, rest zero); block 4
    # holds root_Wp in rows 0:32 of col 0 (padded to 128 wide so the LDW
    # pull-ahead isn't blocked by a narrow stationary)
    rw2 = nc.dram_tensor("rw2", [128, (NPANEL + 1) * 128], mmdt, kind="ExternalInput")
    # all f32 per-partition bias constants in one tensor:
    # cols 0:32 leaf_b, 32:36 int_b, 36:52 leaf_bp (rows 0:8),
    # 52 int_bp (rows 0:16), 53 root_b (rows 0:32), 54 root_bp (row 0)
    cc = nc.dram_tensor("cc", [128, 55], f32, kind="ExternalInput")
    # fp16 output staging: predictions are tanh outputs in [-1,1], so the
    # ~5e-4 fp16 quantization is well inside the error budget; halves the
    # final store drain. Host upcasts to f32.
    out = nc.dram_tensor("out", [NOUT, BC], mmdt, kind="ExternalOutput")

    mm = nc.tensor.matmul

    with tile.TileContext(nc) as tc, ExitStack() as ctx:
        consts = ctx.enter_context(tc.tile_pool(name="consts", bufs=1))
        wpool = ctx.enter_context(tc.tile_pool(name="wpool", bufs=4))
        work = ctx.enter_context(tc.tile_pool(name="work", bufs=18))
        keep = ctx.enter_context(tc.tile_pool(name="keep", bufs=1))
        psum = ctx.enter_context(tc.tile_pool(name="psum", bufs=1, space="PSUM"))

        # --- PE pre-warm: unthrottles the HAM clock gate (PE boots at
        # 1.2 GHz; ~3.4us of sustained activity -> 2.4 GHz). bf16 const
        # APs (preloaded) keep it to one MATMUL per mm (fp32 would run
        # LOW_HIGH 2-pass) and nothing gates the first one.
        warm_st = nc.const_aps.tensor(1.0, (128, 128), bf16)
        warm_mv = nc.const_aps.tensor(1.0, (128, BN), bf16)
        pwarm = psum.tile([128, BN], f32, tag="misc", bufs=1, name="pwarm")
        for _ in range(NWARM):
            mm(pwarm[:], warm_st, warm_mv, start=True, stop=True,
               skip_group_check=True)

        # --- preamble DMA issues. Per-queue ORDER is everything: early
        # 1KB-run transfers sustain only ~140GB/s per queue, and the big
        # 16KB-run panel loads (400+GB/s) crush the other queue's
        # throughput once they start — so everything latency-critical
        # must be fully enqueued on BOTH queues before wp1 is. cc's tiny
        # 220B packets go AFTER the first x chunks (cc isn't needed
        # until the first tanh at ~17us, but in front it delays x-k0 by
        # >1us). scalar: x bn0 chunks + cc (x bn1 issues are deferred
        # into the stream emission — the engine must also be free for
        # leaf tanhs). sync: wave1 + wave2 weight chunks, cw, rw2, then
        # panels 1-3.
        cc_sb = consts.tile([128, 55], f32, name="cc_sb")
        xt_sb = consts.tile([128, KC * BC], mmdt, name="xt_sb")
        wp0 = wpool.tile([128, KC * 1024], mmdt, tag="wpanel", name="wp0")
        for k in range(4):
            nc.scalar.dma_start(
                xt_sb[:, k * BC:k * BC + BN], xt[k * 128:(k + 1) * 128, 0:BN]
            )
        nc.scalar.dma_start(cc_sb[:], cc[:])
        for k in range(4, KC):
            nc.scalar.dma_start(
                xt_sb[:, k * BC:k * BC + BN], xt[k * 128:(k + 1) * 128, 0:BN]
            )
        for k in range(KC):
            nc.sync.dma_start(
                wp0[:, k * 1024:k * 1024 + 512],
                lwh[0, :, k * 1024:k * 1024 + 512],
            )
        for k in range(KC):
            nc.sync.dma_start(
                wp0[:, k * 1024 + 512:(k + 1) * 1024],
                lwh[0, :, k * 1024 + 512:(k + 1) * 1024],
            )
        cw_sb = consts.tile([128, I * 2 * 128], mmdt, name="cw_sb")
        nc.sync.dma_start(cw_sb[:], cw[:])
        rw2_sb = consts.tile([128, (NPANEL + 1) * 128], mmdt, name="rw2_sb")
        nc.sync.dma_start(rw2_sb[:], rw2[:])
        wps = {0: wp0}
        for q in (1, 2, 3):
            wps[q] = wpool.tile([128, KC * 1024], mmdt, tag="wpanel", name=f"wp{q}")
            nc.sync.dma_start(wps[q][:], lwh[q])

        # scalar-engine warm: force the tanh ACT table load during the DMA
        # preamble instead of on the first real activation
        act_warm = work.tile([1, 1], f32, tag="actw", bufs=1, name="act_warm")
        nc.scalar.activation(act_warm[:], pwarm[0:1, 0:1], Tanh)

        # leaf predicts: node i at cols i*BC (+bn*BN); flushed per panel
        lp_sb = keep.tile([8, I * BC], mmdt, name="lp_sb")
        intp_sb = keep.tile([16, BC], mmdt, name="intp_sb")
        rootp_sb = keep.tile([1, BC], mmdt, name="rootp_sb")

        inth = {}      # (panel, bn) -> [128, BN] tile: nodes 4p..4p+3 h^T
        prc1 = psum.tile([128, BN], f32, tag="prcinc", bufs=1, name="prc1")

        # deferred-op FIFO: each entry emits one PE-consuming op (a comb
        # matmul, a root contraction, a flush). Popped twice per leaf
        # group (after the 4th and 8th k-matmul) so producers' tanh
        # latency is always covered by >=0.85us of leaf streaming.
        fifo = []

        def pop_slot():
            if fifo:
                fifo.pop(0)()

        def leaf_mm(wp, gl, k, bn, pg, cols=None):
            c0 = bn * BN if cols is None else cols[0]
            cw_ = BN if cols is None else cols[1]
            mm(
                pg[:],
                wp[:, k * 1024 + gl * 128:k * 1024 + (gl + 1) * 128],
                xt_sb[:, k * BC + c0:k * BC + c0 + cw_],
                start=(k == 0),
                stop=(k == KC - 1),
            )

        def leaf_tanh(p, gl, bn, pg):
            lh = work.tile([128, BN], mmdt, tag="lh", name=f"lh{p}{bn}{gl}")
            nc.scalar.activation(
                lh[:], pg[:], Tanh, bias=cc_sb[:, GPP * p + gl:GPP * p + gl + 1]
            )
            return lh

        def comb_mm(p, il, j, lh, pcomb):
            """Fused internal-trans + leaf-predict matmul.

            pcomb rows 0:32 accumulate node (4p+il)'s hidden
            pre-activation over its two child groups; rows 32:40 pick up
            the group's 4 leaf predict dots via the block-diagonal
            columns (zeros elsewhere)."""
            i = 4 * p + il
            mm(
                pcomb[:],
                cw_sb[:, (2 * i + j) * 128:(2 * i + j + 1) * 128],
                lh[:],
                start=(j == 0),
                stop=(j == 1),
                skip_group_check=True,
            )

        def comb_post(p, il, bn, ith, pcomb):
            i = 4 * p + il
            nc.scalar.activation(
                ith[32 * il:32 * il + 32, :],
                pcomb[0:32, :],
                Tanh,
                bias=cc_sb[32 * il:32 * il + 32, 32 + p:33 + p],
            )
            nc.scalar.activation(
                lp_sb[:, i * BC + bn * BN:i * BC + bn * BN + BN],
                pcomb[32:40, :], Tanh, bias=cc_sb[0:8, 36 + i:37 + i],
            )

        def flush_lp(p, bn=None, irange=(0, 4)):
            i0, i1 = irange
            ni = i1 - i0
            if bn is None:
                nc.sync.dma_start(
                    out[32 * p + 8 * i0:32 * p + 8 * i1, :].rearrange(
                        "(i v) c -> v i c", v=8
                    ),
                    lp_sb[:, (4 * p + i0) * BC:(4 * p + i1) * BC].rearrange(
                        "v (i c) -> v i c", c=BC
                    ),
                )
            else:
                nc.sync.dma_start(
                    out[32 * p + 8 * i0:32 * p + 8 * i1,
                        bn * BN:bn * BN + BN].rearrange("(i v) c -> v i c", v=8),
                    lp_sb[:].rearrange("v (i c) -> v i c", c=BC)[
                        :, 4 * p + i0:4 * p + i1, bn * BN:bn * BN + BN
                    ],
                )

        def root_mm_inc(p, ith):
            """One panel's contribution to the bn=1 root/int-predict
            contraction, accumulated across panels in a persistent bank."""
            mm(
                prc1[:],
                rw2_sb[:, 128 * p:128 * (p + 1)],
                ith[:],
                start=(p == 0),
                stop=(p == NPANEL - 1),
                skip_group_check=True,
            )

        def root_post_mm(bn, prc, rh):
            prp = psum.tile([128, BN], f32, tag="pcomb", bufs=2,
                            name=f"prp{bn}")
            mm(prp[0:128, :], rw2_sb[0:32, NPANEL * 128:(NPANEL + 1) * 128],
               rh[:], start=True, stop=True, skip_group_check=True)
            nc.scalar.activation(
                rootp_sb[0:1, bn * BN:bn * BN + BN], prp[0:1, :], Tanh,
                bias=cc_sb[0:1, 54:55],
            )
            nc.scalar.activation(
                intp_sb[:, bn * BN:bn * BN + BN], prc[32:48, :], Tanh,
                bias=cc_sb[0:16, 52:53],
            )

        # ---- comb closures --------------------------------------------
        pcombs = {}

        def make_comb(p, il, j, bn, lh, ith):
            def run():
                if j == 0:
                    pcombs[(p, il, bn)] = psum.tile(
                        [128, BN], f32, tag="pcomb", bufs=2,
                        name=f"pc{p}{bn}{il}")
                pc = pcombs[(p, il, bn)]
                comb_mm(p, il, j, lh, pc)
                if j == 1:
                    comb_post(p, il, bn, ith, pc)
            return run

        # --- panel 0: pure leaf streams first, k-outer waves for bn=0 so
        # matmuls chase the arriving x/weight chunks. No FIFO pops before
        # bn=1 (cw hasn't landed). The remaining scalar-queue DMA issues
        # (x bn1, cw) are emitted between the tanh batches so the scalar
        # engine's issue backlog never delays a data-gated tanh, and
        # every chunk is issued before any matmul that consumes it.
        def xt_bn1_issue(ks):
            for k in ks:
                nc.scalar.dma_start(
                    xt_sb[:, k * BC + BN:(k + 1) * BC],
                    xt[k * 128:(k + 1) * 128, BN:BC],
                )

        ith00 = keep.tile([128, BN], mmdt, tag="inth00", name="inth00")
        for g0 in (0, 4):
            pgs = [
                psum.tile([128, BN], f32, tag="pg", bufs=4, name=f"pgko{g0}{q}")
                for q in range(4)
            ]
            for k in range(KC):
                for q in range(4):
                    leaf_mm(wp0, g0 + q, k, 0, pgs[q])
            if g0 == 0:
                xt_bn1_issue((0, 1, 2, 3))
            for q in range(4):
                gl = g0 + q
                lh = leaf_tanh(0, gl, 0, pgs[q])
                fifo.append(make_comb(0, gl // 2, gl % 2, 0, lh, ith00))
            if g0 == 0:
                xt_bn1_issue((4, 5, 6, 7))
        inth[(0, 0)] = ith00

        ith01 = keep.tile([128, BN], mmdt, tag="inth01", name="inth01")
        for gl in range(GPP):
            pg = psum.tile([128, BN], f32, tag="pg", bufs=4, name=f"pg0b{gl}")
            for k in range(4):
                leaf_mm(wp0, gl, k, 1, pg)
            pop_slot()
            for k in range(4, KC):
                leaf_mm(wp0, gl, k, 1, pg)
            pop_slot()
            lh = leaf_tanh(0, gl, 1, pg)
            fifo.append(make_comb(0, gl // 2, gl % 2, 1, lh, ith01))
        inth[(0, 1)] = ith01

        fifo.append(lambda: root_mm_inc(0, inth[(0, 1)]))
        fifo.append(lambda: flush_lp(0))

        # --- panels 1..3. Panel 3 runs bn=1 first so its root chain
        # (incremental prc1) closes while bn=0's leaf stream still runs.
        prc0 = {}

        def prc0_partial():
            prc0["t"] = psum.tile([128, BN], f32, tag="misc", bufs=1,
                                  name="prc0")
            for q in range(NPANEL - 1):
                mm(
                    prc0["t"][:],
                    rw2_sb[:, 128 * q:128 * (q + 1)],
                    inth[(q, 0)][:],
                    start=(q == 0),
                    stop=False,
                    skip_group_check=True,
                )

        for p in range(1, NPANEL):
            wp = wps[p]
            bns = (1, 0) if p == NPANEL - 1 else (0, 1)
            for bn in bns:
                ith = keep.tile([128, BN], mmdt, tag=f"inth{p}{bn}",
                                name=f"inth{p}{bn}")
                ngl = 6 if (p == NPANEL - 1 and bn == 0) else 8
                for gl in range(ngl):
                    pg = psum.tile([128, BN], f32, tag="pg", bufs=4,
                                   name=f"pg{p}{bn}{gl}")
                    for k in range(4):
                        leaf_mm(wp, gl, k, bn, pg)
                    pop_slot()
                    for k in range(4, KC):
                        leaf_mm(wp, gl, k, bn, pg)
                    pop_slot()
                    lh = leaf_tanh(p, gl, bn, pg)
                    fifo.append(make_comb(p, gl // 2, gl % 2, bn, lh, ith))
                inth[(p, bn)] = ith

                if p < NPANEL - 1:
                    if bn == 1:
                        fifo.append(
                            lambda p=p: root_mm_inc(p, inth[(p, 1)])
                        )
                        fifo.append(lambda p=p: flush_lp(p))
                elif bn == 1:
                    # close bn1's root chain + store its halves, then open
                    # bn0's root contraction (panels 0..2 ready now). Split
                    # into one-PE-op closures so each tanh's latency hides
                    # under half a leaf group of streaming.
                    rh1 = work.tile([32, BN], mmdt, tag="rh", bufs=2,
                                    name="rh1")

                    def close_a():
                        root_mm_inc(NPANEL - 1, inth[(NPANEL - 1, 1)])

                    def close_b():
                        nc.scalar.activation(rh1[:], prc1[0:32, :], Tanh,
                                             bias=cc_sb[0:32, 53:54])

                    def close_c():
                        root_post_mm(1, prc1, rh1)
                        flush_lp(NPANEL - 1, 1)

                    def close_d():
                        nc.sync.dma_start(
                            out[L:L + I, BN:BC], intp_sb[:, BN:BC]
                        )
                    fifo.append(close_a)
                    fifo.append(close_b)
                    fifo.append(close_c)
                    fifo.append(close_d)
                    fifo.append(prc0_partial)
                else:
                    # endgame: node 15 (groups 6,7) runs as one half +
                    # two quarter column segments; each segment's root
                    # chain is emitted under the next segment's leaf
                    # matmuls, so only the final 128-col chain is serial.
                    # (Full quarters everywhere is worse: 128-col leaf
                    # matmuls are LDWEIGHTS-bound at ~2x the cycle cost,
                    # and the endgame goes scalar-activation-bound.)
                    i3 = 4 * p + 3
                    SEGS = ((0, 256), (256, 128), (384, 128))
                    rh = work.tile([32, BN], mmdt, tag="rh", bufs=2,
                                   name="rh0s")
                    qcombs = {}
                    lhq = {}

                    def q_leaf(si, j):
                        cs, cn = SEGS[si]
                        gl = 6 + j
                        pg = psum.tile([128, cn], f32, tag="pg", bufs=4,
                                       name=f"pgq{si}{j}")
                        for k in range(KC):
                            leaf_mm(wp, gl, k, 0, pg, cols=(cs, cn))
                        lh = work.tile([128, cn], mmdt, tag="lh",
                                       name=f"lhq{si}{j}")
                        nc.scalar.activation(
                            lh[:], pg[:], Tanh,
                            bias=cc_sb[:, GPP * p + gl:GPP * p + gl + 1],
                        )
                        lhq[(si, j)] = lh

                    def q_comb(si):
                        cs, cn = SEGS[si]
                        qcombs[si] = psum.tile([128, cn], f32, tag="pcomb",
                                               bufs=2, name=f"pcq{si}")
                        for j in range(2):
                            mm(
                                qcombs[si][:],
                                cw_sb[:, (2 * i3 + j) * 128:(2 * i3 + j + 1) * 128],
                                lhq.pop((si, j))[:],
                                start=(j == 0),
                                stop=(j == 1),
                                skip_group_check=True,
                            )
                        nc.scalar.activation(
                            ith[96:128, cs:cs + cn], qcombs[si][0:32, :],
                            Tanh, bias=cc_sb[96:128, 32 + p:33 + p],
                        )
                        nc.scalar.activation(
                            lp_sb[:, i3 * BC + cs:i3 * BC + cs + cn],
                            qcombs[si][32:40, :], Tanh,
                            bias=cc_sb[0:8, 36 + i3:37 + i3],
                        )

                    def q_rootA(si):
                        cs, cn = SEGS[si]
                        mm(
                            prc0["t"][:, cs:cs + cn],
                            rw2_sb[:, 128 * (NPANEL - 1):128 * NPANEL],
                            ith[:, cs:cs + cn],
                            start=False,
                            stop=True,
                            skip_group_check=True,
                        )
                        nc.scalar.activation(
                            rh[:, cs:cs + cn], prc0["t"][0:32, cs:cs + cn],
                            Tanh, bias=cc_sb[0:32, 53:54],
                        )

                    def q_rootB(si):
                        cs, cn = SEGS[si]
                        prp = psum.tile([128, cn], f32, tag="pcomb", bufs=2,
                                        name=f"prpq{si}")
                        mm(prp[0:128, :],
                           rw2_sb[0:32, NPANEL * 128:(NPANEL + 1) * 128],
                           rh[:, cs:cs + cn],
                           start=True, stop=True, skip_group_check=True)
                        nc.scalar.activation(
                            rootp_sb[0:1, cs:cs + cn], prp[0:1, :], Tanh,
                            bias=cc_sb[0:1, 54:55],
                        )
                        nc.scalar.activation(
                            intp_sb[:, cs:cs + cn],
                            prc0["t"][32:48, cs:cs + cn],
                            Tanh, bias=cc_sb[0:16, 52:53],
                        )

                    q_leaf(0, 0)
                    pop_slot()          # drains comb(il2) -> then flush
                    q_leaf(0, 1)
                    pop_slot()
                    q_comb(0)
                    q_leaf(1, 0)
                    flush_lp(NPANEL - 1, 0, irange=(0, 3))
                    q_rootA(0)
                    q_leaf(1, 1)
                    q_rootB(0)
                    q_comb(1)
                    q_leaf(2, 0)
                    q_rootA(1)
                    q_leaf(2, 1)
                    q_rootB(1)
                    q_comb(2)
                    q_rootA(2)
                    q_rootB(2)
                    # final flushes, split across both DMA queues
                    flush_lp(NPANEL - 1, 0, irange=(3, 4))
                    nc.scalar.dma_start(out[L:L + I, 0:BN], intp_sb[:, 0:BN])
                    nc.sync.dma_start(out[L + I:NOUT, :], rootp_sb[:])

    nc.compile()
    return nc


def _pack_weights(inp):
    f = np.float32
    f16 = np.float16
    leaf_b = np.asarray(inp["leaf_b"], f)
    int_W = np.asarray(inp["int_W"], f)
    int_b = np.asarray(inp["int_b"], f)
    root_W = np.asarray(inp["root_W"], f)
    root_b = np.asarray(inp["root_b"], f)
    leaf_Wp = np.asarray(inp["leaf_Wp"], f)
    leaf_bp = np.asarray(inp["leaf_bp"], f)
    int_Wp = np.asarray(inp["int_Wp"], f)
    int_bp = np.asarray(inp["int_bp"], f)
    root_Wp = np.asarray(inp["root_Wp"], f)
    root_bp = np.asarray(inp["root_bp"], f)

    w = {}
    lw = np.asarray(inp["leaf_W"], f16).transpose(1, 0, 2).reshape(D, L * H)
    w["lwh"] = np.ascontiguousarray(
        lw.reshape(KC, 128, NPANEL, 1024).transpose(2, 1, 0, 3).reshape(
            NPANEL, 128, KC * 1024
        )
    )

    cw = np.zeros((128, I * 2 * 128), f16)
    for i in range(I):
        for j in range(2):
            base = (2 * i + j) * 128
            # int_W chunk j of node i: rows (c*32+h) = child (4j+c) hidden h
            cw[:, base:base + 32] = int_W[i, 128 * j:128 * (j + 1), :]
            for c in range(4):
                lv = 8 * i + 4 * j + c
                cw[c * 32:(c + 1) * 32, base + 32 + 4 * j + c] = leaf_Wp[lv, :, 0]
    w["cw"] = cw

    rw2 = np.zeros((128, (NPANEL + 1) * 128), f16)
    for q in range(NPANEL):
        rw2[:, 128 * q:128 * q + 32] = root_W[128 * q:128 * (q + 1), :]
        for c in range(4):
            iv = 4 * q + c
            rw2[c * 32:(c + 1) * 32, 128 * q + 32 + 4 * q + c] = int_Wp[iv, :, 0]
    rw2[0:32, NPANEL * 128] = root_Wp[:, 0]
    w["rw2"] = rw2

    cc = np.zeros((128, 55), f)
    cc[:, 0:32] = leaf_b.reshape(32, 128).T       # leaf biases, col=h, part=leaf%...
    cc[:, 32:36] = int_b.reshape(4, 128).T
    cc[0:8, 36:52] = leaf_bp.reshape(16, 8).T
    cc[0:16, 52] = int_bp[:, 0]
    cc[0:32, 53] = root_b
    cc[0, 54] = root_bp[0]
    w["cc"] = cc
    return w


def kernel(**inputs):
    from concourse.bass_utils import run_bass_kernel_spmd

    nc = _CACHE.get("nc")
    if nc is None:
        nc = _CACHE["nc"] = _build_nc()

    x = np.asarray(inputs["x"], np.float32)
    w = _pack_weights(inputs)
    in_maps = []
    for c in range(NCORES):
        m = dict(w)
        m["xt"] = np.ascontiguousarray(x[c * BC:(c + 1) * BC, :].T.astype(np.float16))
        in_maps.append(m)

    res = run_bass_kernel_spmd(nc, in_maps, core_ids=list(range(NCORES)))
    _CACHE["last_res"] = res
    outs = [res.results[c]["out"] for c in range(NCORES)]
    full = np.concatenate([o[:, :, None] for o in outs], axis=1)  # [145, B, 1]
    return full.astype(np.float32)
